# revision 30
# baseline (speedup 1.0000x reference)
"""ConvProduct forward (one-hot 2x2/stride-2 conv) as a Bass/Tile kernel on 8 trn2 cores.

Pure data parallel over batch (8 batches/core).

Host side: x is cast to bf16 and pre-packed DIRECTLY into the matmul's
stationary layout T (partition p = kh*64 + a*32 + kw*16 + cin, free
n = wo*32 + (ho%32), batches concatenated per partition), so there is no
on-device transpose at all. The output is stored as raw PSUM-bank dumps
(bf16) and the host undoes the permutation in numpy.

Per batch:
  - TensorE: each 128-col block c of T is a full [K=128, M=128] stationary
    operand covering 128 pixel columns (wo = 4c..4c+3, ho = 32a..32a+31,
    both a) x all 64 patch values x both kh rows. One bf16 matmul per
    c-block against a block-diagonal one-hot moving operand W [128, 512]
    (cols a*256+o; 1s at rows kh*64+a*32+kw*16+kidx[kh,kw,o]) = the full
    conv for 256 pixels x 256 outputs into half a [128, 1024] PSUM tile,
    1 cycle/row.
  - Evacuation: 2-bank copies PSUM f32 -> st bf16 (cast in the copy),
    ScalarE/VectorE alternating groups.
  - Stores: one 4MB SWDGE DMA per batch pair (32KB descriptors), gen
    deferred so the Pool engine never stalls on evac semaphores; the last
    two batches store separately to shorten the tail.

Why this shape (measured on HW): HWDGE feeds only 4 of 16 SDMA engines and
generates ~10ns/descriptor; SWDGE sprays all 16 engines at ~0.34ns/desc gen
BUT every SWDGE descriptor drags ~2 four-byte bookkeeping packets that
serialize on a subset of engines - so descriptor count, not size, sets the
pace. 4KB+ contiguous descriptors run at full per-engine rate (158ns/4KB).
bf16 I/O halves bytes; outputs quantize x to bf16 (one-hot W exact): max
rel err ~0.5%, inside the 2e-2 gate.
"""
import numpy as np

B, H, Wd, Cin = 64, 128, 128, 16
KH, KW, Cout = 2, 2, 256
Ho, Wo = 64, 64
NCORES = 8
BPC = B // NCORES

_CACHE = {}


def _build_nc():
    import concourse.mybir as mybir
    import concourse.tile as tile
    from concourse import bacc

    f32 = mybir.dt.float32
    bf16 = mybir.dt.bfloat16
    nc = bacc.Bacc("TRN2", target_bir_lowering=False, debug=False)

    F = Wd * Cin  # 2048 els per batch per partition

    # x pre-packed on host into T layout: [128, BPC * 2048] bf16
    x = nc.dram_tensor("x", [128, BPC * F], bf16, kind="ExternalInput")
    w = nc.dram_tensor("w", [128, 2 * Cout], bf16, kind="ExternalInput")
    # one row of 32KB-contiguous partition dumps per batch PAIR
    out = nc.dram_tensor(
        "out", [BPC // 2, 128, 2 * 16 * 512], bf16, kind="ExternalOutput"
    )

    with tile.TileContext(nc) as tc:
        with (
            tc.tile_pool(name="wp", bufs=1) as wp,
            tc.tile_pool(name="qp", bufs=1) as qp,
            tc.tile_pool(name="sp", bufs=4) as sp,
            tc.tile_pool(name="pp", bufs=4, space="PSUM") as pp,
        ):
            w_sb = wp.tile([128, 2 * Cout], bf16)
            nc.sync.dma_start(w_sb[:], w.ap())

            # staged loads: batch 0 arrives in two HWDGE chunks (the sync
            # ring starts generating ~5us before the Pool engine's SWDGE
            # preamble finishes, so the first matmuls start at ~7us instead
            # of ~12us); batch 1 alone and then batches 2-7 in one DMA with
            # 24KB descriptors ride the SWDGE queue.
            t0h = []
            for k in range(2):
                th = qp.tile([128, F // 2], bf16, tag=f"t0{k}")
                nc.sync.dma_start(
                    th[:], x.ap()[:, k * (F // 2):(k + 1) * (F // 2)]
                )
                t0h.append(th)
            t1 = qp.tile([128, F], bf16, tag="t1")
            nc.gpsimd.dma_start(t1[:], x.ap()[:, F:2 * F])
            t27 = qp.tile([128, 6 * F], bf16, tag="t27")
            nc.gpsimd.dma_start(t27[:], x.ap()[:, 2 * F:8 * F])

            def tsl(b, c):
                if b == 0:
                    return t0h[c // 8][:, (c % 8) * 128:(c % 8 + 1) * 128]
                if b == 1:
                    return t1[:, c * 128:(c + 1) * 128]
                base = (b - 2) * F
                return t27[:, base + c * 128:base + (c + 1) * 128]

            HB = 16 * 512  # one batch's st elements

            for b in range(BPC):
                # every half-batch (last batch: quarter-batch) ships the
                # moment its evacuations land, from a small rotating tile:
                # the store queue is fed continuously from the first batch
                # on, so the drain never backloads the tail.
                gpc = 2 if b == BPC - 1 else 4  # psum groups per store chunk
                tag = "stq" if gpc == 2 else "stc"

                for g in range(8):
                    if g % gpc == 0:
                        st = sp.tile([128, gpc * 1024], bf16, tag=tag)
                        chunk_base = (b % 2) * HB + g * 1024
                    ps = pp.tile([128, 1024], f32, tag="ps")
                    for half in range(2):
                        c = g * 2 + half
                        nc.tensor.matmul(
                            ps[:, half * 512:(half + 1) * 512],
                            tsl(b, c),
                            w_sb[:],
                            start=True,
                            stop=True,
                            tile_position=(0, 0),
                        )
                    goff = (g % gpc) * 1024
                    stsl = st[:, goff:goff + 1024]
                    if g % 2 == 0:
                        nc.scalar.copy(stsl, ps[:])
                    else:
                        nc.vector.tensor_copy(stsl, ps[:])
                    if g % gpc == gpc - 1:
                        nc.gpsimd.dma_start(
                            out.ap()[b // 2][:, chunk_base:chunk_base + gpc * 1024],
                            st[:],
                        )

    nc.compile()
    return nc


def _get_nc():
    if "nc" not in _CACHE:
        _CACHE["nc"] = _build_nc()
    return _CACHE["nc"]


def _build_w(kernel_idx: np.ndarray) -> np.ndarray:
    import ml_dtypes

    kidx = np.asarray(kernel_idx).astype(np.int64)
    w = np.zeros((128, 2 * Cout), np.float32)
    o = np.arange(Cout)
    for kh in range(KH):
        for a in range(2):
            for kw in range(KW):
                w[kh * 64 + a * 32 + kw * 16 + kidx[kh, kw], a * Cout + o] = 1.0
    return w.astype(ml_dtypes.bfloat16)


def kernel(x: np.ndarray, kernel_idx: np.ndarray) -> np.ndarray:
    import ml_dtypes
    from concourse.bass_utils import run_bass_kernel_spmd

    xb = np.asarray(x).astype(ml_dtypes.bfloat16)
    # pack to T layout: T[b][kh*64 + a*32 + kw*16 + cin, wo*32 + j]
    #   = x[b, 64a + 2j + kh, 2wo + kw, cin]
    xt = (
        xb.reshape(NCORES, BPC, 2, 32, 2, 64, 2, Cin)  # c, b, a, j, kh, wo, kw, cin
        .transpose(0, 1, 4, 2, 6, 7, 5, 3)             # c, b, kh, a, kw, cin, wo, j
        .reshape(NCORES, BPC, 128, Wd * Cin)
        .transpose(0, 2, 1, 3)                         # c, p, b, f
        .reshape(NCORES, 128, BPC * Wd * Cin)
    )
    xt = np.ascontiguousarray(xt)
    w = _build_w(kernel_idx)
    nc = _get_nc()

    in_maps = [{"x": xt[c], "w": w} for c in range(NCORES)]
    res = run_bass_kernel_spmd(nc, in_maps, core_ids=list(range(NCORES)))
    kernel.last_results = res

    raw = np.concatenate([res.results[c]["out"] for c in range(NCORES)], axis=0)
    # raw[pair, p, b2*8192 + rest] -> per-batch [p, rest]
    raw = raw.reshape(B // 2, 128, 2, 16 * 512).transpose(0, 2, 1, 3)
    # raw[b, wl*32+hl, c*512 + a*256 + o] == out[b, a*32+hl, c*4+wl, o]
    raw = raw.reshape(B, 4, 32, 16, 2, Cout)          # b, wl, hl, c, a, o
    out = raw.transpose(0, 4, 2, 3, 1, 5)             # b, a, hl, c, wl, o
    return np.ascontiguousarray(out.reshape(B, Ho, Wo, Cout), dtype=np.float32)


# revision 31
# speedup vs baseline: 1.0811x; 1.0811x over previous
"""ConvProduct forward (one-hot 2x2/stride-2 conv) as a Bass/Tile kernel on 8 trn2 cores.

Pure data parallel over batch (8 batches/core).

Host side: x is cast to bf16 and pre-packed DIRECTLY into the matmul's
stationary layout T (partition p = kh*64 + a*32 + kw*16 + cin, free
n = wo*32 + (ho%32), batches concatenated per partition), so there is no
on-device transpose at all. The output is stored as raw PSUM-bank dumps
(bf16) and the host undoes the permutation in numpy.

Per batch:
  - TensorE: each 128-col block c of T is a full [K=128, M=128] stationary
    operand covering 128 pixel columns (wo = 4c..4c+3, ho = 32a..32a+31,
    both a) x all 64 patch values x both kh rows. One bf16 matmul per
    c-block against a block-diagonal one-hot moving operand W [128, 512]
    (cols a*256+o; 1s at rows kh*64+a*32+kw*16+kidx[kh,kw,o]) = the full
    conv for 256 pixels x 256 outputs into half a [128, 1024] PSUM tile,
    1 cycle/row.
  - Evacuation: 2-bank copies PSUM f32 -> st bf16 (cast in the copy),
    ScalarE/VectorE alternating groups.
  - Stores: one 4MB SWDGE DMA per batch pair (32KB descriptors), gen
    deferred so the Pool engine never stalls on evac semaphores; the last
    two batches store separately to shorten the tail.

Why this shape (measured on HW): HWDGE feeds only 4 of 16 SDMA engines and
generates ~10ns/descriptor; SWDGE sprays all 16 engines at ~0.34ns/desc gen
BUT every SWDGE descriptor drags ~2 four-byte bookkeeping packets that
serialize on a subset of engines - so descriptor count, not size, sets the
pace. 4KB+ contiguous descriptors run at full per-engine rate (158ns/4KB).
bf16 I/O halves bytes; outputs quantize x to bf16 (one-hot W exact): max
rel err ~0.5%, inside the 2e-2 gate.
"""
import numpy as np

B, H, Wd, Cin = 64, 128, 128, 16
KH, KW, Cout = 2, 2, 256
Ho, Wo = 64, 64
NCORES = 8
BPC = B // NCORES

_CACHE = {}


def _build_nc():
    import concourse.mybir as mybir
    import concourse.tile as tile
    from concourse import bacc

    f32 = mybir.dt.float32
    bf16 = mybir.dt.bfloat16
    nc = bacc.Bacc("TRN2", target_bir_lowering=False, debug=False)

    F = Wd * Cin  # 2048 els per batch per partition

    # x pre-packed on host into T layout: [128, BPC * 2048] bf16
    x = nc.dram_tensor("x", [128, BPC * F], bf16, kind="ExternalInput")
    w = nc.dram_tensor("w", [128, 2 * Cout], bf16, kind="ExternalInput")
    # one row of 32KB-contiguous partition dumps per batch PAIR
    out = nc.dram_tensor(
        "out", [BPC // 2, 128, 2 * 16 * 512], bf16, kind="ExternalOutput"
    )

    with tile.TileContext(nc) as tc:
        with (
            tc.tile_pool(name="wp", bufs=1) as wp,
            tc.tile_pool(name="qp", bufs=1) as qp,
            tc.tile_pool(name="sp", bufs=4) as sp,
            tc.tile_pool(name="pp", bufs=4, space="PSUM") as pp,
        ):
            w_sb = wp.tile([128, 2 * Cout], bf16)
            nc.sync.dma_start(w_sb[:], w.ap())

            # staged loads: single batches first (fast pipeline start),
            # then the remaining six in one DMA with 24KB descriptors
            t0 = qp.tile([128, F], bf16, tag="t0")
            nc.gpsimd.dma_start(t0[:], x.ap()[:, 0:F])
            t1 = qp.tile([128, F], bf16, tag="t1")
            nc.gpsimd.dma_start(t1[:], x.ap()[:, F:2 * F])
            t27 = qp.tile([128, 6 * F], bf16, tag="t27")
            nc.gpsimd.dma_start(t27[:], x.ap()[:, 2 * F:8 * F])

            def tsl(b, c):
                if b == 0:
                    return t0[:, c * 128:(c + 1) * 128]
                if b == 1:
                    return t1[:, c * 128:(c + 1) * 128]
                base = (b - 2) * F
                return t27[:, base + c * 128:base + (c + 1) * 128]

            HB = 16 * 512  # one batch's st elements

            for b in range(BPC):
                # every half-batch (last batch: quarter-batch) ships the
                # moment its evacuations land, from a small rotating tile:
                # the store queue is fed continuously from the first batch
                # on, so the drain never backloads the tail.
                gpc = 2 if b == BPC - 1 else 4  # psum groups per store chunk
                tag = "stq" if gpc == 2 else "stc"

                for g in range(8):
                    if g % gpc == 0:
                        st = sp.tile([128, gpc * 1024], bf16, tag=tag)
                        chunk_base = (b % 2) * HB + g * 1024
                    ps = pp.tile([128, 1024], f32, tag="ps")
                    for half in range(2):
                        c = g * 2 + half
                        nc.tensor.matmul(
                            ps[:, half * 512:(half + 1) * 512],
                            tsl(b, c),
                            w_sb[:],
                            start=True,
                            stop=True,
                            tile_position=(0, 0),
                        )
                    goff = (g % gpc) * 1024
                    stsl = st[:, goff:goff + 1024]
                    if g % 2 == 0:
                        nc.scalar.copy(stsl, ps[:])
                    else:
                        nc.vector.tensor_copy(stsl, ps[:])
                    if g % gpc == gpc - 1:
                        nc.gpsimd.dma_start(
                            out.ap()[b // 2][:, chunk_base:chunk_base + gpc * 1024],
                            st[:],
                        )

    nc.compile()
    return nc


def _get_nc():
    if "nc" not in _CACHE:
        _CACHE["nc"] = _build_nc()
    return _CACHE["nc"]


def _build_w(kernel_idx: np.ndarray) -> np.ndarray:
    import ml_dtypes

    kidx = np.asarray(kernel_idx).astype(np.int64)
    w = np.zeros((128, 2 * Cout), np.float32)
    o = np.arange(Cout)
    for kh in range(KH):
        for a in range(2):
            for kw in range(KW):
                w[kh * 64 + a * 32 + kw * 16 + kidx[kh, kw], a * Cout + o] = 1.0
    return w.astype(ml_dtypes.bfloat16)


def kernel(x: np.ndarray, kernel_idx: np.ndarray) -> np.ndarray:
    import ml_dtypes
    from concourse.bass_utils import run_bass_kernel_spmd

    xb = np.asarray(x).astype(ml_dtypes.bfloat16)
    # pack to T layout: T[b][kh*64 + a*32 + kw*16 + cin, wo*32 + j]
    #   = x[b, 64a + 2j + kh, 2wo + kw, cin]
    xt = (
        xb.reshape(NCORES, BPC, 2, 32, 2, 64, 2, Cin)  # c, b, a, j, kh, wo, kw, cin
        .transpose(0, 1, 4, 2, 6, 7, 5, 3)             # c, b, kh, a, kw, cin, wo, j
        .reshape(NCORES, BPC, 128, Wd * Cin)
        .transpose(0, 2, 1, 3)                         # c, p, b, f
        .reshape(NCORES, 128, BPC * Wd * Cin)
    )
    xt = np.ascontiguousarray(xt)
    w = _build_w(kernel_idx)
    nc = _get_nc()

    in_maps = [{"x": xt[c], "w": w} for c in range(NCORES)]
    res = run_bass_kernel_spmd(nc, in_maps, core_ids=list(range(NCORES)))
    kernel.last_results = res

    raw = np.concatenate([res.results[c]["out"] for c in range(NCORES)], axis=0)
    # raw[pair, p, b2*8192 + rest] -> per-batch [p, rest]
    raw = raw.reshape(B // 2, 128, 2, 16 * 512).transpose(0, 2, 1, 3)
    # raw[b, wl*32+hl, c*512 + a*256 + o] == out[b, a*32+hl, c*4+wl, o]
    raw = raw.reshape(B, 4, 32, 16, 2, Cout)          # b, wl, hl, c, a, o
    out = raw.transpose(0, 4, 2, 3, 1, 5)             # b, a, hl, c, wl, o
    return np.ascontiguousarray(out.reshape(B, Ho, Wo, Cout), dtype=np.float32)


# revision 33
# speedup vs baseline: 1.0966x; 1.0144x over previous
"""ConvProduct forward (one-hot 2x2/stride-2 conv) as a Bass/Tile kernel on 8 trn2 cores.

Pure data parallel over batch (8 batches/core).

Host side: x is cast to bf16 and pre-packed DIRECTLY into the matmul's
stationary layout T (partition p = kh*64 + a*32 + kw*16 + cin, free
n = wo*32 + (ho%32), batches concatenated per partition), so there is no
on-device transpose at all. The output is stored as raw PSUM-bank dumps
(bf16) and the host undoes the permutation in numpy.

Per batch:
  - TensorE: each 128-col block c of T is a full [K=128, M=128] stationary
    operand covering 128 pixel columns (wo = 4c..4c+3, ho = 32a..32a+31,
    both a) x all 64 patch values x both kh rows. One bf16 matmul per
    c-block against a block-diagonal one-hot moving operand W [128, 512]
    (cols a*256+o; 1s at rows kh*64+a*32+kw*16+kidx[kh,kw,o]) = the full
    conv for 256 pixels x 256 outputs into half a [128, 1024] PSUM tile,
    1 cycle/row.
  - Evacuation: 2-bank copies PSUM f32 -> st bf16 (cast in the copy),
    ScalarE/VectorE alternating groups.
  - Stores: one 4MB SWDGE DMA per batch pair (32KB descriptors), gen
    deferred so the Pool engine never stalls on evac semaphores; the last
    two batches store separately to shorten the tail.

Why this shape (measured on HW): HWDGE feeds only 4 of 16 SDMA engines and
generates ~10ns/descriptor; SWDGE sprays all 16 engines at ~0.34ns/desc gen
BUT every SWDGE descriptor drags ~2 four-byte bookkeeping packets that
serialize on a subset of engines - so descriptor count, not size, sets the
pace. 4KB+ contiguous descriptors run at full per-engine rate (158ns/4KB).
bf16 I/O halves bytes; outputs quantize x to bf16 (one-hot W exact): max
rel err ~0.5%, inside the 2e-2 gate.
"""
import numpy as np

B, H, Wd, Cin = 64, 128, 128, 16
KH, KW, Cout = 2, 2, 256
Ho, Wo = 64, 64
NCORES = 8
BPC = B // NCORES

_CACHE = {}


def _build_nc():
    import concourse.mybir as mybir
    import concourse.tile as tile
    from concourse import bacc

    f32 = mybir.dt.float32
    bf16 = mybir.dt.bfloat16
    nc = bacc.Bacc("TRN2", target_bir_lowering=False, debug=False)

    F = Wd * Cin  # 2048 els per batch per partition

    # x pre-packed on host into T layout: [128, BPC * 2048] bf16
    x = nc.dram_tensor("x", [128, BPC * F], bf16, kind="ExternalInput")
    w = nc.dram_tensor("w", [128, 2 * Cout], bf16, kind="ExternalInput")
    # one row of 32KB-contiguous partition dumps per batch PAIR
    out = nc.dram_tensor(
        "out", [BPC // 2, 128, 2 * 16 * 512], bf16, kind="ExternalOutput"
    )

    with tile.TileContext(nc) as tc:
        with (
            tc.tile_pool(name="wp", bufs=1) as wp,
            tc.tile_pool(name="qp", bufs=1) as qp,
            tc.tile_pool(name="sp", bufs=4) as sp,
            tc.tile_pool(name="pp", bufs=4, space="PSUM") as pp,
        ):
            w_sb = wp.tile([128, 2 * Cout], bf16)
            nc.sync.dma_start(w_sb[:], w.ap())

            # staged loads: single batches first (fast pipeline start),
            # then the remaining six in one DMA with 24KB descriptors
            t0 = qp.tile([128, F], bf16, tag="t0")
            nc.gpsimd.dma_start(t0[:], x.ap()[:, 0:F])
            t1 = qp.tile([128, F], bf16, tag="t1")
            nc.gpsimd.dma_start(t1[:], x.ap()[:, F:2 * F])
            # remaining six batches in two 3-batch DMAs: keeps 12KB
            # descriptors but halves the contiguous load block occupying
            # the store queue, so batch 0's store chunks slot in sooner
            t24 = qp.tile([128, 3 * F], bf16, tag="t24")
            nc.gpsimd.dma_start(t24[:], x.ap()[:, 2 * F:5 * F])
            t57 = qp.tile([128, 3 * F], bf16, tag="t57")
            nc.gpsimd.dma_start(t57[:], x.ap()[:, 5 * F:8 * F])

            def tsl(b, c):
                if b == 0:
                    return t0[:, c * 128:(c + 1) * 128]
                if b == 1:
                    return t1[:, c * 128:(c + 1) * 128]
                if b < 5:
                    base = (b - 2) * F
                    return t24[:, base + c * 128:base + (c + 1) * 128]
                base = (b - 5) * F
                return t57[:, base + c * 128:base + (c + 1) * 128]

            HB = 16 * 512  # one batch's st elements

            for b in range(BPC):
                # every half-batch (last batch: quarter-batch) ships the
                # moment its evacuations land, from a small rotating tile:
                # the store queue is fed continuously from the first batch
                # on, so the drain never backloads the tail.
                gpc = 2 if b == BPC - 1 else 4  # psum groups per store chunk
                tag = "stq" if gpc == 2 else "stc"

                for g in range(8):
                    if g % gpc == 0:
                        st = sp.tile([128, gpc * 1024], bf16, tag=tag)
                        chunk_base = (b % 2) * HB + g * 1024
                    ps = pp.tile([128, 1024], f32, tag="ps")
                    for half in range(2):
                        c = g * 2 + half
                        nc.tensor.matmul(
                            ps[:, half * 512:(half + 1) * 512],
                            tsl(b, c),
                            w_sb[:],
                            start=True,
                            stop=True,
                            tile_position=(0, 0),
                        )
                    goff = (g % gpc) * 1024
                    stsl = st[:, goff:goff + 1024]
                    # ScalarE (slightly faster) takes the odd groups so the
                    # last evac gating each store chunk lands sooner
                    if g % 2 == 1:
                        nc.scalar.copy(stsl, ps[:])
                    else:
                        nc.vector.tensor_copy(stsl, ps[:])
                    if g % gpc == gpc - 1:
                        nc.gpsimd.dma_start(
                            out.ap()[b // 2][:, chunk_base:chunk_base + gpc * 1024],
                            st[:],
                        )

    nc.compile()
    return nc


def _get_nc():
    if "nc" not in _CACHE:
        _CACHE["nc"] = _build_nc()
    return _CACHE["nc"]


def _build_w(kernel_idx: np.ndarray) -> np.ndarray:
    import ml_dtypes

    kidx = np.asarray(kernel_idx).astype(np.int64)
    w = np.zeros((128, 2 * Cout), np.float32)
    o = np.arange(Cout)
    for kh in range(KH):
        for a in range(2):
            for kw in range(KW):
                w[kh * 64 + a * 32 + kw * 16 + kidx[kh, kw], a * Cout + o] = 1.0
    return w.astype(ml_dtypes.bfloat16)


def kernel(x: np.ndarray, kernel_idx: np.ndarray) -> np.ndarray:
    import ml_dtypes
    from concourse.bass_utils import run_bass_kernel_spmd

    xb = np.asarray(x).astype(ml_dtypes.bfloat16)
    # pack to T layout: T[b][kh*64 + a*32 + kw*16 + cin, wo*32 + j]
    #   = x[b, 64a + 2j + kh, 2wo + kw, cin]
    xt = (
        xb.reshape(NCORES, BPC, 2, 32, 2, 64, 2, Cin)  # c, b, a, j, kh, wo, kw, cin
        .transpose(0, 1, 4, 2, 6, 7, 5, 3)             # c, b, kh, a, kw, cin, wo, j
        .reshape(NCORES, BPC, 128, Wd * Cin)
        .transpose(0, 2, 1, 3)                         # c, p, b, f
        .reshape(NCORES, 128, BPC * Wd * Cin)
    )
    xt = np.ascontiguousarray(xt)
    w = _build_w(kernel_idx)
    nc = _get_nc()

    in_maps = [{"x": xt[c], "w": w} for c in range(NCORES)]
    res = run_bass_kernel_spmd(nc, in_maps, core_ids=list(range(NCORES)))
    kernel.last_results = res

    raw = np.concatenate([res.results[c]["out"] for c in range(NCORES)], axis=0)
    # raw[pair, p, b2*8192 + rest] -> per-batch [p, rest]
    raw = raw.reshape(B // 2, 128, 2, 16 * 512).transpose(0, 2, 1, 3)
    # raw[b, wl*32+hl, c*512 + a*256 + o] == out[b, a*32+hl, c*4+wl, o]
    raw = raw.reshape(B, 4, 32, 16, 2, Cout)          # b, wl, hl, c, a, o
    out = raw.transpose(0, 4, 2, 3, 1, 5)             # b, a, hl, c, wl, o
    return np.ascontiguousarray(out.reshape(B, Ho, Wo, Cout), dtype=np.float32)
